# revision 31
# baseline (speedup 1.0000x reference)
"""Trainium2 Bass kernel for nn_Attention (B=64, S=2048, RNN=1024, ATT_HID=512).

Data-parallel over batch across 8 NeuronCores, 8 batches per core:
  att_h  = h @ W_h.T + b_h                     (PE, setup)
  scores = w_a . tanh(p_att + att_h)           (split DVE/PE, see below)
  wexp   = mask * exp(scores)                  (ACT exp + DVE fused mul-reduce)
  out    = (sum_s wexp[s] * att_feats[s]) / sum_s wexp[s]   (PE matmuls)

The softmax -> mask -> renormalize of the reference reduces algebraically to
mask*exp(s) / sum(mask*exp(s)); scores are O(1) so exp needs no max-subtraction.

Host-side transforms cut HBM traffic (the original bottleneck) to ~16% of
naive f32:
  1. mask compaction - masked-out positions have weight exactly 0, so their
     p/f rows are never read; each batch is gathered to its live rows and
     padded to a common multiple-of-128 length (the NEFF is compiled for that
     padded length at call time; ~50% density here -> 1152 of 2048 rows);
  2. fp8 (e3m4) for both big streams. f stays e3m4 in SBUF and feeds the PE
     directly as the moving operand of a mixed bf16 x fp8 matmul; p is either
     cast to bf16 in-flight (SWDGE) or consumed raw by the ACT engine.
Measured end-to-end relative error ~1.5e-2 (e3m4 has a 4-bit mantissa; the
f stream contributes ~1.35%, p via tanh ~0.65%), inside the 2e-2 gate.

With DMA halved, the score pipeline is the bottleneck, so batches are split
over two layouts to balance DVE/ACT/PE:
  (a)-batches: position-major p [128 pos, NT*HID], SWDGE cast-DMA to bf16.
     DVE broadcast-add of att_h (2x rate), one big ACT tanh, then per-chunk
     DVE scalar_tensor_tensor (mul by w_a, accumulate) -> scores [128, NT].
  (b)-batches: h-major p [128 h, KCH*NP] (position-permuted). ACT tanh with
     att_h as the per-partition bias vector (no DVE add at all), then the
     w_a contraction runs on the PE (w_aT column x dot chunk), giving score
     rows [1, NP] in PSUM; a tiny SWDGE reshape-DMA redistributes them to
     [128, NT] (the host pre-permutes positions so this is a pure reshape).
Both paths share the tail: exp, mask-mul + row-sum (DVE stt), denominator
matmul + reciprocal, and the weighted att_feats accumulation on the PE
(bf16 weight column x fp8 f tile, fp32 PSUM accumulate).
"""

import sys

import numpy as np

for _p in ("/opt/trn_rl_repo",):
    if _p not in sys.path:
        sys.path.append(_p)

from contextlib import ExitStack

import ml_dtypes

import concourse.bass as bass
from concourse import bacc, mybir, tile
from concourse.bass import ts
from concourse.bass_utils import run_bass_kernel_spmd
from concourse.masks import make_identity

B, S, RNN, HID = 64, 2048, 1024, 512
N_CORES = 8
BL = B // N_CORES
KCH = HID // 128

DT_NP = ml_dtypes.bfloat16
DT8_NP = ml_dtypes.float8_e3m4

# batch -> score-path split: (a) DVE stt, (b) PE contraction. All-(b)
# minimizes total engine work (no DVE multiply-accumulate pass) and keeps the
# PE stream continuous, which matters under the device's activity throttle.
A_LIST = ()
B_LIST = (0, 1, 2, 3, 4, 5, 6, 7)
# software-pipeline depth: batch b's f-accumulation is issued after batch
# b+PIPE's scores, hiding the score-chain latency from the in-order PE stream
PIPE = 2


def build_nc(NP, BL=BL, RNN=RNN, HID=HID, n_cores=N_CORES):
    P = 128
    NT = NP // P           # position chunks of 128
    KC = RNN // P          # contraction chunks for att_h matmul
    NH = max(1, RNN // 512)  # f-matmul column blocks (PSUM bank = 512 f32)
    HW = RNN // NH
    NBLK = -(-NP // 512)   # score-row PSUM blocks
    assert NBLK <= 3, NP
    BLK = [(NP * i // NBLK // 16 * 16) for i in range(NBLK + 1)]
    BLK[-1] = NP
    NA = len(A_LIST)
    NB = len(B_LIST)
    f32 = mybir.dt.float32
    dt = mybir.dt.bfloat16
    dt8 = mybir.dt.float8e3
    Act = mybir.ActivationFunctionType
    Alu = mybir.AluOpType

    nc = bacc.Bacc(
        "TRN2",
        target_bir_lowering=False,
        debug=False,
        enable_asserts=False,
        num_devices=n_cores,
    )

    pa_t = (
        nc.dram_tensor("pa", [NA, P, NT * HID], dt8, kind="ExternalInput").ap()
        if NA
        else None
    )
    pb_t = (
        nc.dram_tensor("pb", [NB, P, KCH * NP], dt8, kind="ExternalInput").ap()
        if NB
        else None
    )
    f_t = nc.dram_tensor("f", [BL, P, NT * RNN], dt8, kind="ExternalInput").ap()
    hT_t = nc.dram_tensor("hT", [P, KC * BL], dt, kind="ExternalInput").ap()
    WhT_t = nc.dram_tensor("WhT", [P, KC * HID], dt, kind="ExternalInput").ap()
    bh_t = nc.dram_tensor("bh", [1, HID], dt, kind="ExternalInput").ap()
    wa_t = nc.dram_tensor("wa", [1, HID], dt, kind="ExternalInput").ap()
    waT_t = nc.dram_tensor("waT", [P, KCH], dt, kind="ExternalInput").ap()
    mk_t = nc.dram_tensor("maskc", [BL, P, NT], f32, kind="ExternalInput").ap()
    out_t = nc.dram_tensor("out", [BL, RNN], f32, kind="ExternalOutput").ap()

    with tile.TileContext(nc) as tc, ExitStack() as ctx:
        const = ctx.enter_context(tc.tile_pool(name="const", bufs=1))

        # hT/WhT arrive host-pre-tiled (contiguous per partition) and load
        # first: they gate the att_h setup chain that everything waits on
        hT_sb = const.tile([P, KC * BL], dt, tag="hT")
        nc.sync.dma_start(hT_sb, hT_t)
        WhT_sb = const.tile([P, KC * HID], dt, tag="WhT")
        nc.sync.dma_start(WhT_sb, WhT_t)
        bh_sb = const.tile([1, HID], dt, tag="bh")
        nc.sync.dma_start(bh_sb, bh_t)
        # small consts ride the idle ACT ring
        wa_sb = const.tile([1, HID], dt, tag="wa")
        nc.scalar.dma_start(wa_sb, wa_t)
        waT_sb = const.tile([P, KCH], dt, tag="waT")
        nc.scalar.dma_start(waT_sb, waT_t)
        mask_sb = const.tile([P, BL * NT], f32, tag="mask")
        nc.scalar.dma_start(
            mask_sb.rearrange("p (b t) -> p b t", b=BL),
            mk_t.rearrange("b p t -> p b t"),
        )
        ones_bl = const.tile([1, BL], dt, tag="ones_bl")
        nc.vector.memset(ones_bl, 1.0)
        ones_col = const.tile([P, 1], f32, tag="ones_col")
        nc.vector.memset(ones_col, 1.0)
        ident8 = const.tile([BL, BL], dt, tag="ident8")
        make_identity(nc, ident8)
        wab_sb = const.tile([P, HID], dt, tag="wab")
        if NA:
            nc.gpsimd.partition_broadcast(wab_sb, wa_sb)

        att_hb = const.tile([BL, HID], dt, tag="att_hb")
        att_rows = const.tile([1, BL * HID], dt, tag="att_rows")
        ahT_sb = const.tile([P, KCH * BL], dt, tag="ahT")

        with tc.tile_pool(name="ps_setup", bufs=1, space="PSUM") as pss:
            ah_ps = pss.tile([BL, HID], f32, tag="ah")
            for c in range(KC):
                nc.tensor.matmul(
                    ah_ps,
                    hT_sb[:, ts(c, BL)],
                    WhT_sb[:, ts(c, HID)],
                    start=(c == 0),
                    stop=False,
                )
            nc.tensor.matmul(ah_ps, ones_bl, bh_sb, start=False, stop=True)
            nc.vector.tensor_copy(att_hb, ah_ps)
            ahT_ps = pss.tile([P, KCH * BL], dt, tag="ahT_ps")
            for c in range(KCH):
                nc.tensor.transpose(
                    ahT_ps[:, ts(c, BL)], att_hb[:, ts(c, P)], ident8
                )
            nc.vector.tensor_copy(ahT_sb, ahT_ps)
        # [BL, HID] rows -> one [1, BL*HID] row on partition 0. One DMA per
        # row: partition-regrouped APs silently corrupt on HW. On sync, NOT
        # scalar: DMA instructions in the ACT stream block the first tanhs.
        if NA:
            for b in range(BL):
                nc.sync.dma_start(
                    att_rows[:, ts(b, HID)], att_hb[b : b + 1, :]
                )

        ps_sc = ctx.enter_context(tc.tile_pool(name="ps_sc", bufs=1, space="PSUM"))
        pdram = ctx.enter_context(tc.tile_pool(name="pdram", bufs=2, space="DRAM"))
        ps_o = ctx.enter_context(tc.tile_pool(name="ps_o", bufs=2, space="PSUM"))
        ppa = ctx.enter_context(tc.tile_pool(name="ppa", bufs=2))
        ppb = ctx.enter_context(tc.tile_pool(name="ppb", bufs=2))
        pdot = ctx.enter_context(tc.tile_pool(name="pdot", bufs=2))
        pab = ctx.enter_context(tc.tile_pool(name="pab", bufs=2))
        py = ctx.enter_context(tc.tile_pool(name="py", bufs=2))
        pf = ctx.enter_context(tc.tile_pool(name="pf", bufs=PIPE + 2))
        psc = ctx.enter_context(tc.tile_pool(name="psc", bufs=PIPE + 3))
        pout = ctx.enter_context(tc.tile_pool(name="pout", bufs=2))

        def load_and_scores(b):
            """DMA batch b's streams and compute w_all/rowsum/rden."""
            ft = pf.tile([P, NT * RNN], dt8, tag="f", name="ft")
            nc.sync.dma_start(ft, f_t[b])

            if b in A_LIST:
                ai = A_LIST.index(b)
                ahb = pab.tile([P, HID], dt, tag="ahb")
                nc.gpsimd.partition_broadcast(ahb, att_rows[:, ts(b, HID)])
                pt = ppa.tile([P, NT * HID], dt, tag="p")
                nc.gpsimd.dma_start(pt, pa_t[ai])  # fp8 -> bf16 in flight
                nc.vector.tensor_add(
                    pt.rearrange("p (i h) -> p i h", i=NT),
                    pt.rearrange("p (i h) -> p i h", i=NT),
                    ahb[:, None, :].broadcast_to([P, NT, HID]),
                )
                # chunked: ACT's 2x bf16 mode doesn't engage on very wide ops
                for j in range(0, NT, 3):
                    jw = min(3, NT - j) * HID
                    nc.scalar.activation(
                        pt[:, j * HID : j * HID + jw],
                        pt[:, j * HID : j * HID + jw],
                        Act.Tanh,
                    )
                s_all = psc.tile([P, NT], f32, tag="s")
                for t in range(NT):
                    scr = py.tile([P, HID], dt, tag="y", name="scr")
                    nc.vector.scalar_tensor_tensor(
                        out=scr,
                        in0=pt[:, ts(t, HID)],
                        scalar=1.0,
                        in1=wab_sb,
                        op0=Alu.mult,
                        op1=Alu.mult,
                        accum_out=s_all[:, t : t + 1],
                    )
            else:
                bi = B_LIST.index(b)
                pbt = ppb.tile([P, KCH * NP], dt, tag="pb")
                nc.gpsimd.dma_start(pbt, pb_t[bi])  # fp8 -> bf16 in flight
                dot = pdot.tile([P, KCH * NP], dt, tag="dot")
                for c in range(KCH):
                    nc.scalar.activation(
                        dot[:, ts(c, NP)],
                        pbt[:, ts(c, NP)],
                        Act.Tanh,
                        bias=ahT_sb[:, c * BL + b : c * BL + b + 1],
                        scale=1.0,
                    )
                scps = [
                    ps_sc.tile(
                        [1, BLK[i + 1] - BLK[i]], f32, tag=f"sc{i}", name=f"sc{i}"
                    )
                    for i in range(NBLK)
                ]
                for i in range(NBLK):
                    for c in range(KCH):
                        nc.tensor.matmul(
                            scps[i],
                            waT_sb[:, c : c + 1],
                            dot[:, c * NP + BLK[i] : c * NP + BLK[i + 1]],
                            start=(c == 0),
                            stop=(c == KCH - 1),
                        )
                srow = psc.tile([1, NP], f32, tag="srow")
                for i in range(NBLK):
                    nc.vector.tensor_copy(
                        srow[:, BLK[i] : BLK[i + 1]], scps[i]
                    )
                # positions were host-permuted so this is a pure reshape;
                # bounce through DRAM (partition-regrouped SBUF APs corrupt
                # on HW; DRAM-side APs are unconstrained). Rides the otherwise
                # idle ACT HWDGE ring: these sit on the score critical path
                # and must not queue behind bulk p/f transfers.
                sd = pdram.tile([1, NP], f32, tag="sd")
                nc.scalar.dma_start(sd, srow)
                s_all = psc.tile([P, NT], f32, tag="s")
                nc.scalar.dma_start(
                    s_all, sd.rearrange("x (p t) -> (x p) t", p=P)
                )

            e_all = psc.tile([P, NT], f32, tag="e")
            nc.scalar.activation(e_all, s_all, Act.Exp)
            w_all = psc.tile([P, NT], dt, tag="w")
            rowsum = psc.tile([P, 1], f32, tag="rs")
            nc.vector.scalar_tensor_tensor(
                out=w_all,
                in0=e_all,
                scalar=1.0,
                in1=mask_sb[:, ts(b, NT)],
                op0=Alu.mult,
                op1=Alu.mult,
                accum_out=rowsum,
            )
            den_ps = ps_sc.tile([1, 1], f32, tag="den")
            nc.tensor.matmul(den_ps, rowsum, ones_col, start=True, stop=True)
            rden = psc.tile([1, 1], f32, tag="rden")
            nc.vector.reciprocal(rden, den_ps)
            return ft, w_all, rden

        def weighted_sum(b, ft, w_all, rden):
            """PE-accumulate w_all . att_feats and store the output row."""
            ohs = [
                ps_o.tile([1, HW], f32, tag=f"o{h}", name=f"oh{h}")
                for h in range(NH)
            ]
            for t in range(NT):
                ibase = t * RNN
                for h in range(NH):
                    nc.tensor.matmul(
                        ohs[h],
                        w_all[:, t : t + 1],
                        ft[:, ibase + h * HW : ibase + (h + 1) * HW],
                        start=(t == 0),
                        stop=(t == NT - 1),
                    )
            out_sb = pout.tile([1, RNN], f32, tag="outrow")
            for h in range(NH):
                nc.vector.tensor_scalar_mul(out_sb[:, ts(h, HW)], ohs[h], rden)
            nc.sync.dma_start(out_t[b : b + 1, :], out_sb)

        # software pipeline: batch b's scores are issued before batch b-1's
        # f-accumulation, so the in-order PE stream always has score matmuls
        # to chew on while the previous batch's weight column is finalized.
        pending = []
        for b in range(BL):
            pending.append((b, *load_and_scores(b)))
            if len(pending) > PIPE:
                weighted_sum(*pending.pop(0))
        for args in pending:
            weighted_sum(*args)

    nc.compile()
    return nc


def build_in_maps(h, att_feats, p_att_feats, att_masks, W_h, b_h, w_a):
    """Shard per core; compact each batch to its mask-live rows, pad to a
    common multiple-of-128 length; convert the two big streams to fp8e3."""
    h = np.asarray(h, dtype=np.float32)
    W_h = np.asarray(W_h, dtype=np.float32)
    b_h = np.asarray(b_h, dtype=np.float32)
    w_a = np.asarray(w_a, dtype=np.float32)
    masks = np.asarray(att_masks)
    live = masks != 0
    n_max = int(live.sum(axis=1).max())
    NT = max(2, -(-n_max // 128))
    NP = NT * 128
    p_all = np.asarray(p_att_feats)
    f_all = np.asarray(att_feats)
    KC = RNN // 128
    # pre-tiled [128, KC*...]: partition p holds k-chunks c at (c*128+p)
    WhT = np.ascontiguousarray(
        W_h.T.reshape(KC, 128, HID).transpose(1, 0, 2).reshape(128, KC * HID)
    ).astype(DT_NP)
    bh = b_h.reshape(1, HID).astype(DT_NP)
    wa = w_a.reshape(1, HID).astype(DT_NP)
    waT = np.ascontiguousarray(w_a.reshape(KCH, 128).T).astype(DT_NP)
    # (b)-path position permutation: host column j holds compacted position
    # (j % NT)*128 + j//NT, so the on-chip [1,NP]->[128,NT] reshape of the
    # score row lands scores back in position-major order.
    perm = (np.arange(NP) % NT) * 128 + np.arange(NP) // NT
    in_maps = []
    for c in range(N_CORES):
        sl = slice(c * BL, (c + 1) * BL)
        pa = np.empty((len(A_LIST), P_, NT * HID), DT8_NP)
        pb = np.empty((len(B_LIST), P_, KCH * NP), DT8_NP)
        fc = np.empty((BL, P_, NT * RNN), DT8_NP)
        mc = np.zeros((BL, NP), np.float32)
        for b in range(BL):
            gb = c * BL + b
            idx = np.flatnonzero(live[gb])
            padidx = np.zeros(NP, np.int64)
            padidx[: len(idx)] = idx
            mc[b, : len(idx)] = 1.0
            pcb = p_all[gb][padidx]          # [NP, HID] f32
            fcb = f_all[gb][padidx]          # [NP, RNN] f32
            fc[b] = (
                fcb.reshape(NT, 128, RNN)
                .transpose(1, 0, 2)
                .reshape(P_, NT * RNN)
                .astype(DT8_NP)
            )
            if b in A_LIST:
                pa[A_LIST.index(b)] = (
                    pcb.reshape(NT, 128, HID)
                    .transpose(1, 0, 2)
                    .reshape(P_, NT * HID)
                    .astype(DT8_NP)
                )
            else:
                pb[B_LIST.index(b)] = (
                    pcb[perm]                 # [NP, HID]
                    .T.reshape(KCH, 128, NP)  # [KCH, 128, NP]
                    .transpose(1, 0, 2)
                    .reshape(P_, KCH * NP)
                    .astype(DT8_NP)
                )
        mcc = mc.reshape(BL, NT, 128).transpose(0, 2, 1)
        in_maps.append(
            {
                **({"pa": pa} if len(A_LIST) else {}),
                **({"pb": pb} if len(B_LIST) else {}),
                "f": fc,
                "hT": np.ascontiguousarray(
                    h[sl].T.reshape(KC, 128, BL)
                    .transpose(1, 0, 2)
                    .reshape(128, KC * BL)
                ).astype(DT_NP),
                "WhT": WhT,
                "bh": bh,
                "wa": wa,
                "waT": waT,
                "maskc": np.ascontiguousarray(mcc),
            }
        )
    return in_maps


P_ = 128
_NC_CACHE = {}


def run(in_maps, trace=False, **kwargs):
    NP = in_maps[0]["maskc"].shape[2] * 128
    if NP not in _NC_CACHE:
        _NC_CACHE[NP] = build_nc(NP)
    return run_bass_kernel_spmd(
        _NC_CACHE[NP], in_maps, core_ids=list(range(N_CORES)), trace=trace, **kwargs
    )


def kernel(h, att_feats, p_att_feats, att_masks, W_h, b_h, w_a, b_a=None):
    # b_a shifts every score equally; softmax normalization cancels it.
    in_maps = build_in_maps(h, att_feats, p_att_feats, att_masks, W_h, b_h, w_a)
    res = run(in_maps, trace=False)
    return np.concatenate([r["out"] for r in res.results], axis=0)


# revision 33
# speedup vs baseline: 1.1390x; 1.1390x over previous
"""Trainium2 Bass kernel for nn_Attention (B=64, S=2048, RNN=1024, ATT_HID=512).

Data-parallel over batch across 8 NeuronCores, 8 batches per core:
  att_h  = h @ W_h.T + b_h                     (PE, setup)
  scores = w_a . tanh(p_att + att_h)           (split DVE/PE, see below)
  wexp   = mask * exp(scores)                  (ACT exp + DVE fused mul-reduce)
  out    = (sum_s wexp[s] * att_feats[s]) / sum_s wexp[s]   (PE matmuls)

The softmax -> mask -> renormalize of the reference reduces algebraically to
mask*exp(s) / sum(mask*exp(s)); scores are O(1) so exp needs no max-subtraction.

Host-side transforms cut HBM traffic (the original bottleneck) to ~16% of
naive f32:
  1. mask compaction - masked-out positions have weight exactly 0, so their
     p/f rows are never read; each batch is gathered to its live rows and
     padded to a common multiple-of-128 length (the NEFF is compiled for that
     padded length at call time; ~50% density here -> 1152 of 2048 rows);
  2. fp8 (e3m4) for both big streams. f stays e3m4 in SBUF and feeds the PE
     directly as the moving operand of a mixed bf16 x fp8 matmul; p is either
     cast to bf16 in-flight (SWDGE) or consumed raw by the ACT engine.
Measured end-to-end relative error ~1.5e-2 (e3m4 has a 4-bit mantissa; the
f stream contributes ~1.35%, p via tanh ~0.65%), inside the 2e-2 gate.

With DMA halved, the score pipeline is the bottleneck, so batches are split
over two layouts to balance DVE/ACT/PE:
  (a)-batches: position-major p [128 pos, NT*HID], SWDGE cast-DMA to bf16.
     DVE broadcast-add of att_h (2x rate), one big ACT tanh, then per-chunk
     DVE scalar_tensor_tensor (mul by w_a, accumulate) -> scores [128, NT].
  (b)-batches: h-major p [128 h, KCH*NP] (position-permuted). ACT tanh with
     att_h as the per-partition bias vector (no DVE add at all), then the
     w_a contraction runs on the PE (w_aT column x dot chunk), giving score
     rows [1, NP] in PSUM; a tiny SWDGE reshape-DMA redistributes them to
     [128, NT] (the host pre-permutes positions so this is a pure reshape).
Both paths share the tail: exp, mask-mul + row-sum (DVE stt), denominator
matmul + reciprocal, and the weighted att_feats accumulation on the PE
(bf16 weight column x fp8 f tile, fp32 PSUM accumulate).
"""

import sys

import numpy as np

for _p in ("/opt/trn_rl_repo",):
    if _p not in sys.path:
        sys.path.append(_p)

from contextlib import ExitStack

import ml_dtypes

import concourse.bass as bass
from concourse import bacc, mybir, tile
from concourse.bass import ts
from concourse.bass_utils import run_bass_kernel_spmd
from concourse.masks import make_identity

B, S, RNN, HID = 64, 2048, 1024, 512
N_CORES = 8
BL = B // N_CORES
KCH = HID // 128

DT_NP = ml_dtypes.bfloat16
DT8_NP = ml_dtypes.float8_e3m4

# batch -> score-path split: (a) DVE stt, (b) PE contraction. All-(b)
# minimizes total engine work (no DVE multiply-accumulate pass) and keeps the
# PE stream continuous, which matters under the device's activity throttle.
A_LIST = ()
B_LIST = (0, 1, 2, 3, 4, 5, 6, 7)
# software-pipeline depth: batch b's f-accumulation is issued after batch
# b+PIPE's scores, hiding the score-chain latency from the in-order PE stream
PIPE = 2


def build_nc(NP, BL=BL, RNN=RNN, HID=HID, n_cores=N_CORES):
    P = 128
    NT = NP // P           # position chunks of 128
    KC = RNN // P          # contraction chunks for att_h matmul
    NH = max(1, RNN // 512)  # f-matmul column blocks (PSUM bank = 512 f32)
    HW = RNN // NH
    NBLK = -(-NP // 512)   # score-row PSUM blocks
    assert NBLK <= 3, NP
    BLK = [(NP * i // NBLK // 16 * 16) for i in range(NBLK + 1)]
    BLK[-1] = NP
    NA = len(A_LIST)
    NB = len(B_LIST)
    f32 = mybir.dt.float32
    dt = mybir.dt.bfloat16
    dt8 = mybir.dt.float8e3
    Act = mybir.ActivationFunctionType
    Alu = mybir.AluOpType

    nc = bacc.Bacc(
        "TRN2",
        target_bir_lowering=False,
        debug=False,
        enable_asserts=False,
        num_devices=n_cores,
    )

    pa_t = (
        nc.dram_tensor("pa", [NA, P, NT * HID], dt8, kind="ExternalInput").ap()
        if NA
        else None
    )
    pb_t = (
        nc.dram_tensor("pb", [NB, P, KCH * NP], dt8, kind="ExternalInput").ap()
        if NB
        else None
    )
    f_t = nc.dram_tensor("f", [BL, P, NT * RNN], dt8, kind="ExternalInput").ap()
    hT_t = nc.dram_tensor("hT", [P, KC * BL], dt, kind="ExternalInput").ap()
    WhT_t = nc.dram_tensor("WhT", [P, KC * HID], dt, kind="ExternalInput").ap()
    bh_t = nc.dram_tensor("bh", [1, HID], dt, kind="ExternalInput").ap()
    wa_t = nc.dram_tensor("wa", [1, HID], dt, kind="ExternalInput").ap()
    waT_t = nc.dram_tensor("waT", [P, KCH], dt, kind="ExternalInput").ap()
    mk_t = nc.dram_tensor("maskc", [BL, P, NT], f32, kind="ExternalInput").ap()
    out_t = nc.dram_tensor("out", [BL, RNN], f32, kind="ExternalOutput").ap()

    with tile.TileContext(nc) as tc, ExitStack() as ctx:
        const = ctx.enter_context(tc.tile_pool(name="const", bufs=1))

        # hT/WhT arrive host-pre-tiled (contiguous per partition) and load
        # first: they gate the att_h setup chain that everything waits on
        hT_sb = const.tile([P, KC * BL], dt, tag="hT")
        nc.sync.dma_start(hT_sb, hT_t)
        WhT_sb = const.tile([P, KC * HID], dt, tag="WhT")
        nc.sync.dma_start(WhT_sb, WhT_t)
        bh_sb = const.tile([1, HID], dt, tag="bh")
        nc.sync.dma_start(bh_sb, bh_t)
        # small consts ride the idle ACT ring
        wa_sb = const.tile([1, HID], dt, tag="wa")
        nc.scalar.dma_start(wa_sb, wa_t)
        waT_sb = const.tile([P, KCH], dt, tag="waT")
        nc.scalar.dma_start(waT_sb, waT_t)
        mask_sb = const.tile([P, BL * NT], f32, tag="mask")
        nc.scalar.dma_start(
            mask_sb.rearrange("p (b t) -> p b t", b=BL),
            mk_t.rearrange("b p t -> p b t"),
        )
        ones_bl = const.tile([1, BL], dt, tag="ones_bl")
        nc.vector.memset(ones_bl, 1.0)
        ones_col = const.tile([P, 1], f32, tag="ones_col")
        nc.vector.memset(ones_col, 1.0)
        ident8 = const.tile([BL, BL], dt, tag="ident8")
        make_identity(nc, ident8)
        wab_sb = const.tile([P, HID], dt, tag="wab")
        if NA:
            nc.gpsimd.partition_broadcast(wab_sb, wa_sb)

        att_hb = const.tile([BL, HID], dt, tag="att_hb")
        att_rows = const.tile([1, BL * HID], dt, tag="att_rows")
        ahT_sb = const.tile([P, KCH * BL], dt, tag="ahT")

        with tc.tile_pool(name="ps_setup", bufs=1, space="PSUM") as pss:
            ah_ps = pss.tile([BL, HID], f32, tag="ah")
            for c in range(KC):
                nc.tensor.matmul(
                    ah_ps,
                    hT_sb[:, ts(c, BL)],
                    WhT_sb[:, ts(c, HID)],
                    start=(c == 0),
                    stop=False,
                )
            nc.tensor.matmul(ah_ps, ones_bl, bh_sb, start=False, stop=True)
            nc.vector.tensor_copy(att_hb, ah_ps)
            ahT_ps = pss.tile([P, KCH * BL], dt, tag="ahT_ps")
            for c in range(KCH):
                nc.tensor.transpose(
                    ahT_ps[:, ts(c, BL)], att_hb[:, ts(c, P)], ident8
                )
            nc.vector.tensor_copy(ahT_sb, ahT_ps)
        # [BL, HID] rows -> one [1, BL*HID] row on partition 0. One DMA per
        # row: partition-regrouped APs silently corrupt on HW. On sync, NOT
        # scalar: DMA instructions in the ACT stream block the first tanhs.
        if NA:
            for b in range(BL):
                nc.sync.dma_start(
                    att_rows[:, ts(b, HID)], att_hb[b : b + 1, :]
                )

        ps_sc = ctx.enter_context(tc.tile_pool(name="ps_sc", bufs=1, space="PSUM"))
        pdram = ctx.enter_context(tc.tile_pool(name="pdram", bufs=2, space="DRAM"))
        ps_o = ctx.enter_context(tc.tile_pool(name="ps_o", bufs=2, space="PSUM"))
        ppa = ctx.enter_context(tc.tile_pool(name="ppa", bufs=2))
        ppb = ctx.enter_context(tc.tile_pool(name="ppb", bufs=2))
        pdot = ctx.enter_context(tc.tile_pool(name="pdot", bufs=2))
        pab = ctx.enter_context(tc.tile_pool(name="pab", bufs=2))
        py = ctx.enter_context(tc.tile_pool(name="py", bufs=2))
        pf = ctx.enter_context(tc.tile_pool(name="pf", bufs=PIPE + 2))
        psc = ctx.enter_context(tc.tile_pool(name="psc", bufs=PIPE + 3))
        pout = ctx.enter_context(tc.tile_pool(name="pout", bufs=2))

        def load_and_scores(b):
            """DMA batch b's streams and compute w_all/rowsum/rden."""
            ft = pf.tile([P, NT * RNN], dt8, tag="f", name="ft")
            nc.sync.dma_start(ft, f_t[b])

            if b in A_LIST:
                ai = A_LIST.index(b)
                ahb = pab.tile([P, HID], dt, tag="ahb")
                nc.gpsimd.partition_broadcast(ahb, att_rows[:, ts(b, HID)])
                pt = ppa.tile([P, NT * HID], dt, tag="p")
                nc.gpsimd.dma_start(pt, pa_t[ai])  # fp8 -> bf16 in flight
                nc.vector.tensor_add(
                    pt.rearrange("p (i h) -> p i h", i=NT),
                    pt.rearrange("p (i h) -> p i h", i=NT),
                    ahb[:, None, :].broadcast_to([P, NT, HID]),
                )
                # chunked: ACT's 2x bf16 mode doesn't engage on very wide ops
                for j in range(0, NT, 3):
                    jw = min(3, NT - j) * HID
                    nc.scalar.activation(
                        pt[:, j * HID : j * HID + jw],
                        pt[:, j * HID : j * HID + jw],
                        Act.Tanh,
                    )
                s_all = psc.tile([P, NT], f32, tag="s")
                for t in range(NT):
                    scr = py.tile([P, HID], dt, tag="y", name="scr")
                    nc.vector.scalar_tensor_tensor(
                        out=scr,
                        in0=pt[:, ts(t, HID)],
                        scalar=1.0,
                        in1=wab_sb,
                        op0=Alu.mult,
                        op1=Alu.mult,
                        accum_out=s_all[:, t : t + 1],
                    )
            else:
                bi = B_LIST.index(b)
                # raw fp8 on the SP HWDGE ring: ACT reads fp8 at the same
                # rate as bf16, so an in-flight cast would only waste SWDGE
                pbt = ppb.tile([P, KCH * NP], dt8, tag="pb")
                nc.sync.dma_start(pbt, pb_t[bi])
                dot = pdot.tile([P, KCH * NP], dt, tag="dot")
                for c in range(KCH):
                    nc.scalar.activation(
                        dot[:, ts(c, NP)],
                        pbt[:, ts(c, NP)],
                        Act.Tanh,
                        bias=ahT_sb[:, c * BL + b : c * BL + b + 1],
                        scale=1.0,
                    )
                scps = [
                    ps_sc.tile(
                        [1, BLK[i + 1] - BLK[i]], f32, tag=f"sc{i}", name=f"sc{i}"
                    )
                    for i in range(NBLK)
                ]
                for i in range(NBLK):
                    for c in range(KCH):
                        nc.tensor.matmul(
                            scps[i],
                            waT_sb[:, c : c + 1],
                            dot[:, c * NP + BLK[i] : c * NP + BLK[i + 1]],
                            start=(c == 0),
                            stop=(c == KCH - 1),
                        )
                srow = psc.tile([1, NP], f32, tag="srow")
                for i in range(NBLK):
                    nc.vector.tensor_copy(
                        srow[:, BLK[i] : BLK[i + 1]], scps[i]
                    )
                # positions were host-permuted so this is a pure reshape;
                # bounce through DRAM (partition-regrouped SBUF APs corrupt
                # on HW; DRAM-side APs are unconstrained). Rides the otherwise
                # SWDGE ring, which carries nothing else in the all-(b)
                # config (p/f ride the SP HWDGE ring), so no head-blocking.
                sd = pdram.tile([1, NP], f32, tag="sd")
                nc.gpsimd.dma_start(sd, srow)
                s_all = psc.tile([P, NT], f32, tag="s")
                nc.gpsimd.dma_start(
                    s_all, sd.rearrange("x (p t) -> (x p) t", p=P)
                )

            e_all = psc.tile([P, NT], f32, tag="e")
            nc.scalar.activation(e_all, s_all, Act.Exp)
            w_all = psc.tile([P, NT], dt, tag="w")
            rowsum = psc.tile([P, 1], f32, tag="rs")
            nc.vector.scalar_tensor_tensor(
                out=w_all,
                in0=e_all,
                scalar=1.0,
                in1=mask_sb[:, ts(b, NT)],
                op0=Alu.mult,
                op1=Alu.mult,
                accum_out=rowsum,
            )
            den_ps = ps_sc.tile([1, 1], f32, tag="den")
            nc.tensor.matmul(den_ps, rowsum, ones_col, start=True, stop=True)
            rden = psc.tile([1, 1], f32, tag="rden")
            nc.vector.reciprocal(rden, den_ps)
            return ft, w_all, rden

        def weighted_sum(b, ft, w_all, rden):
            """PE-accumulate w_all . att_feats and store the output row."""
            ohs = [
                ps_o.tile([1, HW], f32, tag=f"o{h}", name=f"oh{h}")
                for h in range(NH)
            ]
            for t in range(NT):
                ibase = t * RNN
                for h in range(NH):
                    nc.tensor.matmul(
                        ohs[h],
                        w_all[:, t : t + 1],
                        ft[:, ibase + h * HW : ibase + (h + 1) * HW],
                        start=(t == 0),
                        stop=(t == NT - 1),
                    )
            out_sb = pout.tile([1, RNN], f32, tag="outrow")
            for h in range(NH):
                nc.vector.tensor_scalar_mul(out_sb[:, ts(h, HW)], ohs[h], rden)
            nc.sync.dma_start(out_t[b : b + 1, :], out_sb)

        # software pipeline: batch b's scores are issued before batch b-1's
        # f-accumulation, so the in-order PE stream always has score matmuls
        # to chew on while the previous batch's weight column is finalized.
        pending = []
        for b in range(BL):
            pending.append((b, *load_and_scores(b)))
            if len(pending) > PIPE:
                weighted_sum(*pending.pop(0))
        for args in pending:
            weighted_sum(*args)

    nc.compile()
    return nc


def build_in_maps(h, att_feats, p_att_feats, att_masks, W_h, b_h, w_a):
    """Shard per core; compact each batch to its mask-live rows, pad to a
    common multiple-of-128 length; convert the two big streams to fp8e3."""
    h = np.asarray(h, dtype=np.float32)
    W_h = np.asarray(W_h, dtype=np.float32)
    b_h = np.asarray(b_h, dtype=np.float32)
    w_a = np.asarray(w_a, dtype=np.float32)
    masks = np.asarray(att_masks)
    live = masks != 0
    n_max = int(live.sum(axis=1).max())
    NT = max(2, -(-n_max // 128))
    NP = NT * 128
    p_all = np.asarray(p_att_feats)
    f_all = np.asarray(att_feats)
    KC = RNN // 128
    # pre-tiled [128, KC*...]: partition p holds k-chunks c at (c*128+p)
    WhT = np.ascontiguousarray(
        W_h.T.reshape(KC, 128, HID).transpose(1, 0, 2).reshape(128, KC * HID)
    ).astype(DT_NP)
    bh = b_h.reshape(1, HID).astype(DT_NP)
    wa = w_a.reshape(1, HID).astype(DT_NP)
    waT = np.ascontiguousarray(w_a.reshape(KCH, 128).T).astype(DT_NP)
    # (b)-path position permutation: host column j holds compacted position
    # (j % NT)*128 + j//NT, so the on-chip [1,NP]->[128,NT] reshape of the
    # score row lands scores back in position-major order.
    perm = (np.arange(NP) % NT) * 128 + np.arange(NP) // NT
    in_maps = []
    for c in range(N_CORES):
        sl = slice(c * BL, (c + 1) * BL)
        pa = np.empty((len(A_LIST), P_, NT * HID), DT8_NP)
        pb = np.empty((len(B_LIST), P_, KCH * NP), DT8_NP)
        fc = np.empty((BL, P_, NT * RNN), DT8_NP)
        mc = np.zeros((BL, NP), np.float32)
        for b in range(BL):
            gb = c * BL + b
            idx = np.flatnonzero(live[gb])
            padidx = np.zeros(NP, np.int64)
            padidx[: len(idx)] = idx
            mc[b, : len(idx)] = 1.0
            pcb = p_all[gb][padidx]          # [NP, HID] f32
            fcb = f_all[gb][padidx]          # [NP, RNN] f32
            fc[b] = (
                fcb.reshape(NT, 128, RNN)
                .transpose(1, 0, 2)
                .reshape(P_, NT * RNN)
                .astype(DT8_NP)
            )
            if b in A_LIST:
                pa[A_LIST.index(b)] = (
                    pcb.reshape(NT, 128, HID)
                    .transpose(1, 0, 2)
                    .reshape(P_, NT * HID)
                    .astype(DT8_NP)
                )
            else:
                pb[B_LIST.index(b)] = (
                    pcb[perm]                 # [NP, HID]
                    .T.reshape(KCH, 128, NP)  # [KCH, 128, NP]
                    .transpose(1, 0, 2)
                    .reshape(P_, KCH * NP)
                    .astype(DT8_NP)
                )
        mcc = mc.reshape(BL, NT, 128).transpose(0, 2, 1)
        in_maps.append(
            {
                **({"pa": pa} if len(A_LIST) else {}),
                **({"pb": pb} if len(B_LIST) else {}),
                "f": fc,
                "hT": np.ascontiguousarray(
                    h[sl].T.reshape(KC, 128, BL)
                    .transpose(1, 0, 2)
                    .reshape(128, KC * BL)
                ).astype(DT_NP),
                "WhT": WhT,
                "bh": bh,
                "wa": wa,
                "waT": waT,
                "maskc": np.ascontiguousarray(mcc),
            }
        )
    return in_maps


P_ = 128
_NC_CACHE = {}


def run(in_maps, trace=False, **kwargs):
    NP = in_maps[0]["maskc"].shape[2] * 128
    if NP not in _NC_CACHE:
        _NC_CACHE[NP] = build_nc(NP)
    return run_bass_kernel_spmd(
        _NC_CACHE[NP], in_maps, core_ids=list(range(N_CORES)), trace=trace, **kwargs
    )


def kernel(h, att_feats, p_att_feats, att_masks, W_h, b_h, w_a, b_a=None):
    # b_a shifts every score equally; softmax normalization cancels it.
    in_maps = build_in_maps(h, att_feats, p_att_feats, att_masks, W_h, b_h, w_a)
    res = run(in_maps, trace=False)
    return np.concatenate([r["out"] for r in res.results], axis=0)
